# revision 8
# baseline (speedup 1.0000x reference)
"""Weighted per-class dice loss on 8 trn2 NeuronCores (batch-sharded).

Per core (one batch element), pixels viewed as [128, 4096], all math bf16.
For each class c the kernel needs den_c = <W, pred_c> + <W, mask_c> and
inter_c = <W, pred_c*mask_c>, mask_c = (L==c).  Frobenius products are
computed on the tensor engine via the trace trick: for 128-col chunks k,
accumulate W_k^T @ X_k into a [128,128] PSUM mat; its diagonal sums to
<W, X>.  Diagonals are pulled with an identity-masked scalar_tensor_tensor
accumulate into accs[:, c]; a ones-vector matmul folds accs -> [1, 38].

Per-class engine work is balanced between DVE and PE by two variants:
  s-variant  (12 classes): DVE mask (tensor_scalar is_equal, 4x mode),
      s = pred+mask, mp = pred*mask (tensor_tensor, 2x mode); PE streams
      s (den) and mp (inter): 64 matmuls.
  3-stream   (7 classes): DVE mask + mp only; PE streams pred and mask
      into den (one accumulation group) plus mp: 96 matmuls.
pred_c is DMA'd f32, split across the two HWDGE queues (sync + scalar
engines), cast to bf16 on the scalar engine, double-buffered so
DMA / ACT / DVE / PE pipeline across classes.
Host sums the 8 cores' partials and applies the dice formula.
"""

import ml_dtypes
import numpy as np

import concourse.bass as bass
from concourse import mybir
from concourse.bass_utils import run_bass_kernel_spmd

C = 19
P = 128
FCOL_FULL = 4096
SMOOTH = 1.0

F = mybir.dt.float32
BF = mybir.dt.bfloat16

# classes where PE takes the extra stream (c % 3 == 0): 7 of 19
def _is3(c):
    return c % 3 == 0


def build_nc(fcol: int = FCOL_FULL) -> bass.Bass:
    kch = fcol // P
    nc = bass.Bass()
    pred = nc.dram_tensor("pred", [C, P, fcol], F, kind="ExternalInput")
    tgt = nc.dram_tensor("target", [2, P, fcol], F, kind="ExternalInput")
    ident_d = nc.dram_tensor("ident", [P, P], BF, kind="ExternalInput")
    partials = nc.dram_tensor("partials", [1, 2 * C], F, kind="ExternalOutput")

    mult = mybir.AluOpType.mult
    add = mybir.AluOpType.add
    is_eq = mybir.AluOpType.is_equal

    from contextlib import ExitStack

    _es = ExitStack()
    with _es:
        def sb(name, shape, dt):
            return _es.enter_context(nc.sbuf_tensor(name, shape, dt))

        lf = sb("lf", [P, fcol], F); wf = sb("wf", [P, fcol], F)
        lb = sb("lb", [P, fcol], BF); wb = sb("wb", [P, fcol], BF)
        pf0 = sb("pf0", [P, fcol], F); pf1 = sb("pf1", [P, fcol], F)
        pf2 = sb("pf2", [P, fcol], F)
        pb0 = sb("pb0", [P, fcol], BF); pb1 = sb("pb1", [P, fcol], BF)
        maskb = sb("maskb", [P, fcol], BF)
        s0 = sb("s0", [P, fcol], BF); s1 = sb("s1", [P, fcol], BF)
        mp0 = sb("mp0", [P, fcol], BF); mp1 = sb("mp1", [P, fcol], BF)
        identb = sb("identb", [P, P], BF)
        junk = sb("junk", [P, P], BF)
        accs = sb("accs", [P, 2 * C], F)
        ones = sb("ones", [P, 1], F)
        outsb = sb("outsb", [1, 2 * C], F)
        den0 = _es.enter_context(nc.psum_tensor("den0", [P, P], F))
        den1 = _es.enter_context(nc.psum_tensor("den1", [P, P], F))
        int0 = _es.enter_context(nc.psum_tensor("int0", [P, P], F))
        int1 = _es.enter_context(nc.psum_tensor("int1", [P, P], F))
        ps = _es.enter_context(nc.psum_tensor("ps", [1, 2 * C], F))

        def sem(name):
            return _es.enter_context(nc.semaphore(name))

        tgt_sem = sem("tgt_sem")
        dsem0 = sem("dsem0"); dsem1 = sem("dsem1"); dsem2 = sem("dsem2")
        act_sem = sem("act_sem")  # 2 prologue casts + 1 per class
        vs_sem = sem("vs_sem")    # 2 per class (s|mask, mp)
        ve_sem = sem("ve_sem")    # 2 per class extracts; +1 final copy
        pe_sem = sem("pe_sem")    # 1 per class; +1 final fold
        block = _es.enter_context(nc.Block())
        pfs = [pf0, pf1, pf2]; pbs = [pb0, pb1]
        ss = [s0, s1]; mps = [mp0, mp1]
        dens = [den0, den1]; ints = [int0, int1]
        dsems = [dsem0, dsem1, dsem2]
        half = fcol // 2

        def pb_free_wait(eng, c):
            """Wait until pb[c%2] (written by cast c) is free to overwrite:
            consumer of class c-2 is done."""
            j = c - 2
            if _is3(j):
                eng.wait_ge(pe_sem, j + 1)   # PE streamed pb directly
            else:
                eng.wait_ge(vs_sem, 2 * j + 2)  # DVE mp done

        @block.sync
        def _(sync: bass.BassEngine):
            sync.dma_start(out=lf[:, 0:half], in_=tgt[0][:, 0:half]
                           ).then_inc(tgt_sem, 16)
            sync.dma_start(out=wf[:, 0:half], in_=tgt[1][:, 0:half]
                           ).then_inc(tgt_sem, 16)
            sync.dma_start(out=identb[:], in_=ident_d[:]).then_inc(tgt_sem, 16)
            for c in range(C):
                if c >= 3:
                    # pf[c%3] free once ACT finished cast of iter c-3
                    sync.wait_ge(act_sem, 2 + (c - 3) + 1)
                sync.dma_start(
                    out=pfs[c % 3][:, 0:half], in_=pred[c][:, 0:half]
                ).then_inc(dsems[c % 3], 16)
            sync.wait_ge(ve_sem, 2 * C + 1)
            sync.dma_start(out=partials[:], in_=outsb[:]).then_inc(tgt_sem, 16)

        @block.scalar
        def _(scalar: bass.BassEngine):
            scalar.dma_start(out=lf[:, half:fcol], in_=tgt[0][:, half:fcol]
                             ).then_inc(tgt_sem, 16)
            scalar.dma_start(out=wf[:, half:fcol], in_=tgt[1][:, half:fcol]
                             ).then_inc(tgt_sem, 16)
            for c in range(C):
                if c >= 3:
                    scalar.wait_ge(act_sem, 2 + (c - 3) + 1)
                scalar.dma_start(
                    out=pfs[c % 3][:, half:fcol], in_=pred[c][:, half:fcol]
                ).then_inc(dsems[c % 3], 16)
                if c == 0:
                    scalar.wait_ge(tgt_sem, 64)
                    scalar.copy(out=lb[:], in_=lf[:]).then_inc(act_sem, 1)
                    scalar.copy(out=wb[:], in_=wf[:]).then_inc(act_sem, 1)
                scalar.wait_ge(dsems[c % 3], 32 * (c // 3 + 1))
                if c >= 2:
                    pb_free_wait(scalar, c)
                scalar.copy(out=pbs[c % 2][:], in_=pfs[c % 3][:]).then_inc(
                    act_sem, 1)

        @block.vector
        def _(vector: bass.BassEngine):
            vector.memset(ones[:], 1.0)
            for c in range(C):
                vector.wait_ge(act_sem, 3 + c)
                if c >= 2:
                    # s/mp[c%2] free once PE finished class c-2
                    vector.wait_ge(pe_sem, c - 1)
                pb = pbs[c % 2]
                if _is3(c):
                    # mask straight into s-buffer; PE streams it for den
                    vector.tensor_scalar(
                        out=ss[c % 2][:], in0=lb[:], scalar1=float(c),
                        scalar2=None, op0=is_eq).then_inc(vs_sem, 1)
                    vector.tensor_tensor(
                        out=mps[c % 2][:], in0=pb[:], in1=ss[c % 2][:],
                        op=mult).then_inc(vs_sem, 1)
                else:
                    vector.tensor_scalar(
                        out=maskb[:], in0=lb[:], scalar1=float(c),
                        scalar2=None, op0=is_eq)
                    vector.tensor_tensor(
                        out=ss[c % 2][:], in0=pb[:], in1=maskb[:], op=add
                    ).then_inc(vs_sem, 1)
                    vector.tensor_tensor(
                        out=mps[c % 2][:], in0=pb[:], in1=maskb[:], op=mult
                    ).then_inc(vs_sem, 1)
                if c >= 1:
                    vector.wait_ge(pe_sem, c)
                    j = c - 1
                    vector.scalar_tensor_tensor(
                        out=junk[:], in0=dens[j % 2][:], scalar=1.0,
                        in1=identb[:], op0=mult, op1=mult,
                        accum_out=accs[:, j : j + 1]).then_inc(ve_sem, 1)
                    vector.scalar_tensor_tensor(
                        out=junk[:], in0=ints[j % 2][:], scalar=1.0,
                        in1=identb[:], op0=mult, op1=mult,
                        accum_out=accs[:, C + j : C + j + 1]).then_inc(ve_sem, 1)
            vector.wait_ge(pe_sem, C)
            j = C - 1
            vector.scalar_tensor_tensor(
                out=junk[:], in0=dens[j % 2][:], scalar=1.0,
                in1=identb[:], op0=mult, op1=mult,
                accum_out=accs[:, j : j + 1]).then_inc(ve_sem, 1)
            vector.scalar_tensor_tensor(
                out=junk[:], in0=ints[j % 2][:], scalar=1.0,
                in1=identb[:], op0=mult, op1=mult,
                accum_out=accs[:, C + j : C + j + 1]).then_inc(ve_sem, 1)
            vector.wait_ge(pe_sem, C + 1)
            vector.tensor_copy(out=outsb[:], in_=ps[:]).then_inc(ve_sem, 1)

        @block.tensor
        def _(tensor: bass.BassEngine):
            tensor.wait_ge(act_sem, 2)
            for c in range(C):
                den = dens[c % 2]; intm = ints[c % 2]
                if c >= 2:
                    tensor.wait_ge(ve_sem, 2 * (c - 2) + 2)
                if _is3(c):
                    # den = sum_k W_k^T pred_k + sum_k W_k^T mask_k
                    tensor.wait_ge(act_sem, 3 + c)  # pb ready
                    for k in range(kch):
                        ch = slice(k * P, (k + 1) * P)
                        tensor.matmul(den[:], wb[:, ch], pbs[c % 2][:, ch],
                                      start=(k == 0), stop=False)
                    tensor.wait_ge(vs_sem, 2 * c + 1)  # mask (in s-buffer)
                    for k in range(kch):
                        ch = slice(k * P, (k + 1) * P)
                        tensor.matmul(den[:], wb[:, ch], ss[c % 2][:, ch],
                                      start=False, stop=(k == kch - 1))
                else:
                    tensor.wait_ge(vs_sem, 2 * c + 1)  # s ready
                    for k in range(kch):
                        ch = slice(k * P, (k + 1) * P)
                        tensor.matmul(den[:], wb[:, ch], ss[c % 2][:, ch],
                                      start=(k == 0), stop=(k == kch - 1))
                tensor.wait_ge(vs_sem, 2 * c + 2)  # mp ready
                last = None
                for k in range(kch):
                    ch = slice(k * P, (k + 1) * P)
                    last = tensor.matmul(intm[:], wb[:, ch],
                                         mps[c % 2][:, ch],
                                         start=(k == 0), stop=(k == kch - 1))
                last.then_inc(pe_sem, 1)
            tensor.wait_ge(ve_sem, 2 * C)
            tensor.matmul(ps[:], ones[:], accs[:], start=True, stop=True
                          ).then_inc(pe_sem, 1)

    return nc


def _combine(parts: np.ndarray) -> np.ndarray:
    tot = parts.astype(np.float64).sum(axis=0)
    den, inter = tot[0:C], tot[C : 2 * C]
    dice = (2.0 * inter + SMOOTH) / (den + SMOOTH)
    loss = np.sum(1.0 - dice) / C
    return np.asarray(loss, dtype=np.float32)


def make_in_maps(pred: np.ndarray, target: np.ndarray):
    B = pred.shape[0]
    n = pred.shape[2] * pred.shape[3]
    fcol = n // P
    pred_r = np.ascontiguousarray(pred.reshape(B, C, P, fcol).astype(np.float32))
    tgt_r = np.ascontiguousarray(target.reshape(B, 2, P, fcol).astype(np.float32))
    ident = np.eye(P, dtype=ml_dtypes.bfloat16)
    return [
        {"pred": pred_r[i], "target": tgt_r[i], "ident": ident}
        for i in range(B)
    ], fcol


def kernel(pred: np.ndarray, target: np.ndarray) -> np.ndarray:
    in_maps, fcol = make_in_maps(pred, target)
    nc = build_nc(fcol)
    res = run_bass_kernel_spmd(nc, in_maps, list(range(len(in_maps)))).results
    parts = np.stack([r["partials"].reshape(-1) for r in res])
    return _combine(parts)
